# revision 24
# baseline (speedup 1.0000x reference)
"""Transformer encoder layer (LN -> MHA -> residual -> LN -> MLP -> residual)
on 8 Trainium2 NeuronCores.

Sharding: token-parallel over the 4096 (batch*seq) tokens, 512 query-tokens
per core; the 4 cores sharing a batch each redundantly compute the full
2048-token K/V for that batch, so no collectives are needed.

All matmul operands are bf16 (accumulation stays f32 in PSUM): this enables
the PE's Fast Weight Load path (fp32 weights pay a serial ~107ns LDWEIGHTS
per matmul) and halves weight DMA traffic.  K/V weights stay resident in
SBUF so the per-kv-chunk loop re-reads them for free.

On-chip layout: activations are kept feature-major ("transposed", [d, token])
so every matmul contracts along the partition dim with weights in natural
[d_in, d_out] layout.  Softmax is computed unnormalized (scores are bounded,
so plain exp is numerically safe and algebraically identical); the denominator
comes for free from a ones-column appended to V, and the division is applied
to the tiny per-head attention accumulator.

LayerNorm gains/biases are folded into the following projections on the host
(exact algebra: (g*xhat+b) @ W = xhat @ (diag(g) W) + b @ W).
"""

import numpy as np
import ml_dtypes

import concourse.bass as bass
import concourse.mybir as mybir
from concourse import bacc
from concourse.tile import TileContext
from concourse.bass_utils import run_bass_kernel_spmd
from concourse.masks import make_identity

F32 = mybir.dt.float32
BF16 = mybir.dt.bfloat16
AF = mybir.ActivationFunctionType
ALU = mybir.AluOpType

B, S, D = 2, 2048, 1024
H, HD = 16, 64
DFF = 4 * D
NCORES = 8
QT = 512           # query tokens per core
NCHUNK = S // 512  # kv chunks of 512 tokens
EPS = 1e-5


def _ln_to_hT(nc, lnp, psM, cpool_refs, mr_dram, xT_dram, col0, hT):
    """LayerNorm 512 tokens with HOST-precomputed per-token stats
    (mr_dram rows: 0 = -mu*rstd, 1 = rstd, bf16): broadcast the rows across
    partitions via rank-1 PE matmuls, then hT = xT * rs + mr in transposed
    space over the DMA'd x^T bits."""
    ident, eps, ones128 = cpool_refs
    mr_row = lnp.tile([1, 512], BF16, tag="ln_mr_row")
    nc.sync.dma_start(out=mr_row, in_=mr_dram[0:1, col0:col0 + 512])
    rs_row = lnp.tile([1, 512], BF16, tag="ln_rs_row")
    nc.sync.dma_start(out=rs_row, in_=mr_dram[1:2, col0:col0 + 512])
    bc_ps = psM.tile([128, 1024], F32, tag="big", name="bc_ps")
    nc.tensor.matmul(bc_ps[:, 0:512], ones128, mr_row, start=True, stop=True)
    nc.tensor.matmul(bc_ps[:, 512:1024], ones128, rs_row, start=True, stop=True)
    mr_bc = lnp.tile([128, 512], BF16, tag="mr")
    nc.vector.tensor_copy(mr_bc, bc_ps[:, 0:512])
    rs_bc = lnp.tile([128, 512], BF16, tag="rs")
    nc.vector.tensor_copy(rs_bc, bc_ps[:, 512:1024])
    for dt in range(8):
        nc.sync.dma_start(
            out=hT[:, dt, :],
            in_=xT_dram[dt * 128:(dt + 1) * 128, col0:col0 + 512],
        )
        nc.vector.tensor_mul(hT[:, dt, :], hT[:, dt, :], rs_bc)
        nc.vector.tensor_add(hT[:, dt, :], hT[:, dt, :], mr_bc)


def _build():
    nc = bacc.Bacc(None, target_bir_lowering=False)

    MRB = nc.declare_dram_parameter("mrb", [2, S], BF16, isOutput=False)
    MRQ = nc.declare_dram_parameter("mrq", [2, QT], BF16, isOutput=False)
    XBT = nc.declare_dram_parameter("xbt", [D, S], BF16, isOutput=False)
    XQT = nc.declare_dram_parameter("xqt", [D, QT], BF16, isOutput=False)
    XQ32 = nc.declare_dram_parameter("xq32", [QT, D], F32, isOutput=False)
    WQ = nc.declare_dram_parameter("wq", [D, D], BF16, isOutput=False)
    WK = nc.declare_dram_parameter("wk", [D, D], BF16, isOutput=False)
    WV = nc.declare_dram_parameter("wv", [D, D], BF16, isOutput=False)
    WO = nc.declare_dram_parameter("wo", [D, D], BF16, isOutput=False)
    W1 = nc.declare_dram_parameter("w1", [D, DFF], BF16, isOutput=False)
    W2 = nc.declare_dram_parameter("w2", [DFF, D], BF16, isOutput=False)
    BQ = nc.declare_dram_parameter("bq", [D], F32, isOutput=False)
    BK = nc.declare_dram_parameter("bk", [D], F32, isOutput=False)
    BV = nc.declare_dram_parameter("bv", [D], F32, isOutput=False)
    BO = nc.declare_dram_parameter("bo", [D], F32, isOutput=False)
    B1 = nc.declare_dram_parameter("b1", [DFF], F32, isOutput=False)
    B2 = nc.declare_dram_parameter("b2", [D], F32, isOutput=False)
    Y = nc.declare_dram_parameter("y", [QT, D], F32, isOutput=True)

    with TileContext(nc) as tc:
        with (
            tc.tile_pool(name="const", bufs=1) as cpool,
            tc.tile_pool(name="accp", bufs=1) as accp,
        ):
            ident = cpool.tile([128, 128], F32)
            make_identity(nc, ident)
            eps = cpool.tile([128, 1], F32)
            nc.vector.memset(eps, EPS)
            ones64 = cpool.tile([1, 64], BF16)
            nc.vector.memset(ones64, 1.0)
            onecol = cpool.tile([128, 1], BF16)
            nc.vector.memset(onecol, 1.0)
            ones128 = cpool.tile([1, 128], BF16)
            nc.vector.memset(ones128, 1.0)
            bqT = cpool.tile([128, 8], F32)
            nc.sync.dma_start(out=bqT, in_=BQ[:].rearrange("(t p) -> p t", p=128))
            bkT = cpool.tile([128, 8], F32)
            nc.sync.dma_start(out=bkT, in_=BK[:].rearrange("(t p) -> p t", p=128))
            b1T = cpool.tile([128, 32], F32)
            nc.sync.dma_start(out=b1T, in_=B1[:].rearrange("(t p) -> p t", p=128))
            bv_bc = cpool.tile([128, D], F32)
            nc.sync.dma_start(out=bv_bc, in_=BV[:].partition_broadcast(128))
            bo_bc = cpool.tile([128, D], F32)
            nc.sync.dma_start(out=bo_bc, in_=BO[:].partition_broadcast(128))
            b2_bc = cpool.tile([128, D], F32)
            nc.sync.dma_start(out=b2_bc, in_=B2[:].partition_broadcast(128))
            cpool_refs = (ident, eps, ones128)

            acc = accp.tile([128, 8, 512], F32)   # unnormalized attn^T, head pairs stacked
            accd = accp.tile([33, 8, 512], F32)   # softmax denominators: row 0 = even head, row 32 = odd

            # ---- projections + attention, streamed over kv chunks ----
            with (
                tc.tile_pool(name="qp", bufs=1) as qp,
                tc.tile_pool(name="lnp", bufs=2) as lnp,
                tc.tile_pool(name="hTp", bufs=2) as hTp,
                tc.tile_pool(name="ktp", bufs=2) as ktp,
                tc.tile_pool(name="vp", bufs=2) as vp,
                tc.tile_pool(name="wsm", bufs=2) as wsm,
                tc.tile_pool(name="pp", bufs=2) as ppl,
                tc.tile_pool(name="psM", bufs=3, space="PSUM") as psM,
            ):
                # Q projection from the core's own tokens; Q weights are
                # DMA'd first so the PE can start before the big resident loads
                hqT = qp.tile([128, 8, 512], BF16)
                wq_blocks = []
                for hb in range(2):
                    wqc = wsm.tile([128, 8, 512], BF16, tag="w", name=f"wqc{hb}")
                    nc.sync.dma_start(
                        out=wqc,
                        in_=WQ[:, hb * 512:(hb + 1) * 512].rearrange(
                            "(t p) n -> p t n", p=128
                        ),
                    )
                    wq_blocks.append(wqc)
                _ln_to_hT(nc, lnp, psM, cpool_refs, MRQ, XQT, 0, hqT)
                # resident K/V weights (bf16, 16KB/partition each; scoped to
                # phase B so the space frees for the MLP phase)
                wk_sb = qp.tile([128, 8, D], BF16)
                nc.sync.dma_start(out=wk_sb, in_=WK[:].rearrange("(t p) n -> p t n", p=128))
                wv_sb = qp.tile([128, 8, D], BF16)
                nc.sync.dma_start(out=wv_sb, in_=WV[:].rearrange("(t p) n -> p t n", p=128))
                Q_sb = qp.tile([128, 8, 512], BF16)  # Q^T [hd, q]
                for hb in range(2):
                    wqc = wq_blocks[hb]
                    for ho in range(4):
                        ht = hb * 4 + ho
                        psq = psM.tile([128, 1024], F32, tag="big", name=f"psq{ht}")
                        for dt in range(8):
                            nc.tensor.matmul(
                                psq[:, 0:512], wqc[:, dt, ho * 128:(ho + 1) * 128],
                                hqT[:, dt, :],
                                start=(dt == 0), stop=(dt == 7),
                            )
                        nc.vector.tensor_scalar_add(
                            Q_sb[:, ht, :], psq[:, 0:512], bqT[:, ht:ht + 1]
                        )

                hT = hTp.tile([128, 8, 512], BF16, tag="hT", name="hT_pre")
                _ln_to_hT(nc, lnp, psM, cpool_refs, MRB, XBT, 0, hT)
                for kc in range(NCHUNK):
                    # K^T chunk [hd, 512]
                    KT = ktp.tile([128, 8, 512], BF16, tag="KT")
                    for ht in range(8):
                        psk = psM.tile([128, 1024], F32, tag="big", name=f"psk{ht}")
                        for dt in range(8):
                            nc.tensor.matmul(
                                psk[:, 0:512], wk_sb[:, dt, ht * 128:(ht + 1) * 128],
                                hT[:, dt, :],
                                start=(dt == 0), stop=(dt == 7),
                            )
                        nc.vector.tensor_scalar_add(
                            KT[:, ht, :], psk[:, 0:512], bkT[:, ht:ht + 1]
                        )

                    # V chunk, natural layout [token, st, head, hd]; both
                    # output halves share each stationary hT slice (one
                    # LDWEIGHTS per dt) and land in the two banks of one tile
                    V = vp.tile([128, 4, 16, 64], BF16, tag="V")
                    for st in range(4):
                        psv = psM.tile([128, 1024], F32, tag="big", name=f"psv{st}")
                        for dt in range(8):
                            for hc in range(2):
                                nc.tensor.matmul(
                                    psv[:, hc * 512:(hc + 1) * 512],
                                    hT[:, dt, st * 128:(st + 1) * 128],
                                    wv_sb[:, dt, hc * 512:(hc + 1) * 512],
                                    start=(dt == 0),
                                    stop=(dt == 7),
                                )
                        nc.vector.tensor_add(
                            V[:, st, :, :],
                            psv.rearrange("p (h d) -> p h d", h=16),
                            bv_bc.rearrange("p (h d) -> p h d", h=16),
                        )

                    # LayerNorm for the NEXT chunk overlaps this chunk's
                    # attention (the apply runs on DVE under attention's PE work)
                    if kc + 1 < NCHUNK:
                        hT_next = hTp.tile([128, 8, 512], BF16, tag="hT", name=f"hT_{kc+1}")
                        _ln_to_hT(nc, lnp, psM, cpool_refs, MRB, XBT, (kc + 1) * 512, hT_next)
                    else:
                        hT_next = None

                    # attention: head pairs (2j at partitions 0-63, 2j+1 at
                    # 64-127) issue row-tiled score matmuls that run
                    # CONCURRENTLY on the two halves of the PE array.
                    for j in range(H // 2):
                        P = ppl.tile([128, 4, 2, 512], BF16, tag="P")
                        for kt in range(4):
                            pss = psM.tile([128, 1024], F32, tag="big", name=f"pss{j}_{kt}")
                            nc.tensor.matmul(
                                pss[:, 0:512],
                                KT[0:64, j, kt * 128:(kt + 1) * 128],
                                Q_sb[0:64, j, :],
                                start=True, stop=True,
                            )
                            nc.tensor.matmul(
                                pss[:, 512:1024],
                                KT[64:128, j, kt * 128:(kt + 1) * 128],
                                Q_sb[64:128, j, :],
                                start=True, stop=True,
                            )
                            nc.scalar.activation(
                                P[:, kt, :, :], pss, AF.Exp, scale=0.125
                            )
                        # the two heads' attn@V run as concurrent column
                        # tiles (out partitions 0-63 / 64-127 of one bank);
                        # denominators via rank-1-column matmuls, also
                        # col-tiled pairwise (positions (0,0) and (0,32))
                        psa = psM.tile([128, 512], F32, tag="psa", bufs=2, name=f"psa{j}")
                        for kt in range(4):
                            nc.tensor.matmul(
                                psa[0:64, :], V[:, kt, 2 * j, :], P[:, kt, 0, :],
                                start=(kt == 0), stop=(kt == 3),
                            )
                            nc.tensor.matmul(
                                psa[64:128, :], V[:, kt, 2 * j + 1, :], P[:, kt, 1, :],
                                start=(kt == 0), stop=(kt == 3),
                            )
                        den = psM.tile([128, 1024], F32, tag="big", name=f"den{j}")
                        for kt in range(4):
                            nc.tensor.matmul(
                                den[0:1, 0:512], onecol, P[:, kt, 0, :],
                                start=(kt == 0), stop=(kt == 3),
                            )
                            nc.tensor.matmul(
                                den[32:33, 0:512], onecol, P[:, kt, 1, :],
                                start=(kt == 0), stop=(kt == 3),
                            )
                        if kc == 0:
                            nc.vector.tensor_copy(acc[:, j, :], psa)
                            nc.vector.tensor_copy(accd[0:1, j, :], den[0:1, 0:512])
                            nc.vector.tensor_copy(accd[32:33, j, :], den[32:33, 0:512])
                        else:
                            nc.vector.tensor_add(acc[:, j, :], acc[:, j, :], psa)
                            nc.vector.tensor_add(
                                accd[0:1, j, :], accd[0:1, j, :], den[0:1, 0:512]
                            )
                            nc.vector.tensor_add(
                                accd[32:33, j, :], accd[32:33, j, :], den[32:33, 0:512]
                            )
                    hT = hT_next

            # ---- softmax normalization + out-projection + residual ----
            with tc.tile_pool(name="x2p", bufs=1) as x2p:
              x2 = x2p.tile([128, 4, D], F32)  # post-attention residual stream
              with (
                  tc.tile_pool(name="h2p", bufs=1) as h2p,
                  tc.tile_pool(name="gp", bufs=1) as gp,
              ):
                h2T = h2p.tile([128, 8, 512], BF16)
                G = gp.tile([128, 32, 512], BF16)
                with (
                    tc.tile_pool(name="attnp", bufs=1) as attnp,
                    tc.tile_pool(name="dsm", bufs=4) as dsm,
                    tc.tile_pool(name="lnp2", bufs=2) as lnp2,
                    tc.tile_pool(name="psRB", bufs=2, space="PSUM") as psRB,
                    tc.tile_pool(name="xqp", bufs=1) as xqp,
                    tc.tile_pool(name="dtmp", bufs=4) as dtmp,
                    tc.tile_pool(name="psO", bufs=4, space="PSUM") as psO,
                    tc.tile_pool(name="psT2", bufs=2, space="PSUM") as psT2,
                ):
                    xq_sb = xqp.tile([128, 4, D], F32)
                    nc.sync.dma_start(
                        out=xq_sb, in_=XQ32[:].rearrange("(t p) n -> p t n", p=128)
                    )
                    wo_sb = xqp.tile([128, 8, D], BF16)
                    nc.sync.dma_start(
                        out=wo_sb, in_=WO[:].rearrange("(t p) n -> p t n", p=128)
                    )
                    attn128 = attnp.tile([128, 8, 512], BF16)
                    for j in range(H // 2):
                        # stage the denominator rows contiguously (approx-recip
                        # mishandles offset APs; exact reciprocal costs ~2.7us);
                        # broadcast 1/den to the pair's partition halves via
                        # col-tiled rank-1 matmuls (f32r moving, no bf16 cast)
                        dcA = dsm.tile([1, 512], F32, tag="dcA")
                        nc.vector.tensor_copy(dcA, accd[0:1, j, :])
                        dcB = dsm.tile([1, 512], F32, tag="dcB")
                        nc.vector.tensor_copy(dcB, accd[32:33, j, :])
                        rA = dsm.tile([1, 512], F32, tag="rA")
                        nc.vector.reciprocal_approx_fast(rA, dcA)
                        rB = dsm.tile([1, 512], F32, tag="rB")
                        nc.vector.reciprocal_approx_fast(rB, dcB)
                        rAb = dsm.tile([1, 512], BF16, tag="rAb")
                        nc.vector.tensor_copy(rAb, rA)
                        rBb = dsm.tile([1, 512], BF16, tag="rBb")
                        nc.vector.tensor_copy(rBb, rB)
                        rb_ps = psRB.tile([128, 512], F32, tag="rb")
                        nc.tensor.matmul(rb_ps[0:64, :], ones64, rAb, start=True, stop=True)
                        nc.tensor.matmul(rb_ps[64:128, :], ones64, rBb, start=True, stop=True)
                        nc.vector.tensor_mul(attn128[:, j, :], acc[:, j, :], rb_ps)
                    for qt in range(4):
                        po = [psO.tile([128, 512], F32, tag="psO", name=f"po{qt}_{c}") for c in range(2)]
                        for j in range(8):
                            for c in range(2):
                                nc.tensor.matmul(
                                    po[c], attn128[:, j, qt * 128:(qt + 1) * 128],
                                    wo_sb[:, j, c * 512:(c + 1) * 512],
                                    start=(j == 0), stop=(j == 7),
                                )
                        for c in range(2):
                            t1 = dtmp.tile([128, 512], F32, tag="t1")
                            nc.vector.tensor_add(
                                t1, po[c], bo_bc[:, c * 512:(c + 1) * 512]
                            )
                            nc.vector.tensor_add(
                                x2[:, qt, c * 512:(c + 1) * 512],
                                t1,
                                xq_sb[:, qt, c * 512:(c + 1) * 512],
                            )
                        # LN2 for this token block, interleaved under out-proj
                        xt = x2[:, qt, :]
                        stats = lnp2.tile([128, 2, 6], F32, tag="ln_st")
                        nc.vector.bn_stats(stats[:, 0, :], xt[:, 0:512])
                        nc.vector.bn_stats(stats[:, 1, :], xt[:, 512:1024])
                        mv = lnp2.tile([128, 2], F32, tag="ln_mv")
                        nc.vector.bn_aggr(mv, stats)
                        sd = lnp2.tile([128, 1], F32, tag="ln_sd")
                        nc.scalar.activation(sd, mv[:, 1:2], AF.Sqrt, bias=eps[:, 0:1])
                        rstd = lnp2.tile([128, 1], F32, tag="ln_rs")
                        nc.vector.reciprocal_approx_fast(rstd, sd)
                        hh = lnp2.tile([128, D], F32, tag="ln_h")
                        nc.vector.tensor_scalar(
                            hh, xt, mv[:, 0:1], rstd[:, 0:1], ALU.subtract, ALU.mult
                        )
                        for dt in range(8):
                            pst = psT2.tile([128, 128], F32, tag="tp")
                            nc.tensor.transpose(pst, hh[:, dt * 128:(dt + 1) * 128], ident)
                            nc.vector.tensor_copy(h2T[:, dt, qt * 128:(qt + 1) * 128], pst)

                # ---- MLP + residual ----
                with (
                    tc.tile_pool(name="wfp", bufs=3) as wfp,
                    tc.tile_pool(name="w2p", bufs=6) as w2p,
                    tc.tile_pool(name="yp", bufs=2) as yp,
                ):
                  with (
                      tc.tile_pool(name="psF", bufs=4, space="PSUM") as psF,
                  ):
                      # MLP1: gelu(h2 @ w1 + b1), transposed output [dff, q]
                      for fb in range(8):
                          w1c = wfp.tile([128, 8, 512], BF16, tag="w1")
                          nc.sync.dma_start(
                              out=w1c,
                              in_=W1[:, fb * 512:(fb + 1) * 512].rearrange(
                                  "(t p) n -> p t n", p=128
                              ),
                          )
                          for fo in range(4):
                              ft = fb * 4 + fo
                              psf = psF.tile([128, 512], F32, tag="psF")
                              for dt in range(8):
                                  nc.tensor.matmul(
                                      psf, w1c[:, dt, fo * 128:(fo + 1) * 128],
                                      h2T[:, dt, :],
                                      start=(dt == 0), stop=(dt == 7),
                                  )
                              nc.scalar.activation(
                                  G[:, ft, :], psf, AF.Gelu, bias=b1T[:, ft:ft + 1]
                              )

                  # MLP2: y = G^T @ w2 + b2 + x2
                  with tc.tile_pool(name="psY", bufs=4, space="PSUM") as psY:
                    for c in range(2):
                      py = [psY.tile([128, 512], F32, tag="psY", name=f"py{c}_{i}") for i in range(4)]
                      for ft in range(32):
                          w2t = w2p.tile([128, 512], BF16, tag="w2")
                          nc.sync.dma_start(
                              out=w2t,
                              in_=W2[ft * 128:(ft + 1) * 128, c * 512:(c + 1) * 512],
                          )
                          for qt in range(4):
                              nc.tensor.matmul(
                                  py[qt], G[:, ft, qt * 128:(qt + 1) * 128], w2t,
                                  start=(ft == 0), stop=(ft == 31),
                              )
                      for qt in range(4):
                          t1 = yp.tile([128, 512], F32, tag="yt1")
                          nc.vector.tensor_add(
                              t1, py[qt], b2_bc[:, c * 512:(c + 1) * 512]
                          )
                          yt = yp.tile([128, 512], F32, tag="yt2")
                          nc.vector.tensor_add(
                              yt, t1, x2[:, qt, c * 512:(c + 1) * 512]
                          )
                          nc.sync.dma_start(
                              out=Y[qt * 128:(qt + 1) * 128, c * 512:(c + 1) * 512],
                              in_=yt,
                          )

    nc.compile()
    return nc


_NC = None


def _get_nc():
    global _NC
    if _NC is None:
        _NC = _build()
    return _NC


def _make_in_maps(inputs):
    f32 = lambda a: np.ascontiguousarray(np.asarray(a, dtype=np.float32))
    bf16 = lambda a: np.ascontiguousarray(
        np.asarray(a, dtype=np.float32).astype(ml_dtypes.bfloat16)
    )
    x = f32(inputs["x"])
    ln1_g, ln1_b = f32(inputs["ln1_g"]), f32(inputs["ln1_b"])
    ln2_g, ln2_b = f32(inputs["ln2_g"]), f32(inputs["ln2_b"])
    wq, wk, wv, wo = (f32(inputs[k]) for k in ("wq", "wk", "wv", "wo"))
    w1, w2 = f32(inputs["w1"]), f32(inputs["w2"])
    bq, bk, bv, bo = (f32(inputs[k]) for k in ("bq", "bk", "bv", "bo"))
    b1, b2 = f32(inputs["b1"]), f32(inputs["b2"])

    # Fold LayerNorm affine params into the following projections (exact).
    common = {
        "wq": bf16(ln1_g[:, None] * wq),
        "wk": bf16(ln1_g[:, None] * wk),
        "wv": bf16(ln1_g[:, None] * wv),
        "wo": bf16(wo),
        "w1": bf16(ln2_g[:, None] * w1),
        "w2": bf16(w2),
        "bq": f32(bq + ln1_b @ wq),
        "bk": f32(bk + ln1_b @ wk),
        "bv": f32(bv + ln1_b @ wv),
        "bo": f32(bo),
        "b1": f32(b1 + ln2_b @ w1),
        "b2": f32(b2),
    }
    # host-side LayerNorm-1 statistics (input-only dependent): rows are
    # [-mu*rstd; rstd] per token, quantized to bf16 for the on-chip broadcast
    xb32 = x.astype(np.float32).astype(ml_dtypes.bfloat16).astype(np.float32)
    mu = xb32.mean(axis=2)
    var = ((xb32 - mu[:, :, None]) ** 2).mean(axis=2)
    rstd = 1.0 / np.sqrt(var + 1e-5)
    mrs = [
        np.ascontiguousarray(
            np.stack([-mu[b] * rstd[b], rstd[b]]).astype(ml_dtypes.bfloat16)
        )
        for b in range(B)
    ]
    in_maps = []
    for c in range(NCORES):
        b = c // 4
        qoff = (c % 4) * QT
        m = dict(common)
        m["mrb"] = mrs[b]
        m["mrq"] = mrs[b][:, qoff:qoff + QT]
        m["xbt"] = bf16(x[b].T)
        m["xqt"] = bf16(x[b, qoff:qoff + QT].T)
        m["xq32"] = f32(x[b, qoff:qoff + QT])
        in_maps.append(m)
    return in_maps


def kernel(x, ln1_g, ln1_b, wq, bq, wk, bk, wv, bv, wo, bo, w1, b1, w2, b2, ln2_g, ln2_b):
    inputs = dict(
        x=x, ln1_g=ln1_g, ln1_b=ln1_b, wq=wq, bq=bq, wk=wk, bk=bk, wv=wv, bv=bv,
        wo=wo, bo=bo, w1=w1, b1=b1, w2=w2, b2=b2, ln2_g=ln2_g, ln2_b=ln2_b,
    )
    in_maps = _make_in_maps(inputs)
    nc = _get_nc()
    res = run_bass_kernel_spmd(nc, in_maps, core_ids=list(range(NCORES)))

    y = np.empty((B, S, D), dtype=np.float32)
    for c in range(NCORES):
        b = c // 4
        qoff = (c % 4) * QT
        y[b, qoff:qoff + QT] = res.results[c]["y"]
    return y
